# revision 61
# baseline (speedup 1.0000x reference)
"""AttentionHead kernel for Trainium2, 8 NeuronCores, data-parallel over batch.

Problem (fixed shapes):
    input_tensor [8, 2048, 1024] f32, attention_mask [8, 2048] int64 (0/1),
    Wq/Wk/Wv [1024, 128] f32, bq/bk/bv [128] f32.
    out = softmax(mask(Q @ K^T / sqrt(2048))) @ V    -> [8, 2048, 128] f32

Sharding: one batch element per core (B == n_cores == 8). No collectives.

v2+ schedule (vs v1): the ~20us ScalarE exp stream is the phase-C pacer,
so the DMA order is arranged to start it as early as possible:
  - XKV (compacted keys, bf16, 2.36MB) streams FIRST on the sync queue
    (c0 single, then chunk-pairs, then c7 - few transfers stay under the
    ~8-deep DMA-semaphore rotation); KT/VT accumulate per chunk under the
    DMA shadow.
  - XT ships fp8-e3m4 (2MB instead of 4MB bf16) query-block-major
    [128, block, chunk, 512] so QT block t unblocks as its DMA lands;
    blocks 0,1 ride the scalar queue behind the weights, blocks 2,3
    trail on sync. Wk/Wq also ship e3m4 at 16x scale (one packed blob);
    the 1/16 unfolds for free in kt's Identity-cast scale and in the exp
    scale (together with 1/sqrt(S)). Measured rel err 1.04e-2 vs the 2e-2
    gate (host numpy emulation predicts the device error exactly).
  - Scores/exp for q-blocks (0,1) start once KT + QT0/QT1 exist; QT2/QT3
    and the 9 V-transposes interleave 3-PE-ops-per-exp-tile under the
    pair-0 exp stream (r_ops), then the steady pair-1 loop runs exp-paced
    with AV as in v1.
  - Trail after the last exp: av(2,8) + the pd(2)/pd(3) chains (denom
    matmul -> reciprocal -> gpsimd broadcast -> mul -> out DMA) go first;
    the deferred block-3 AV matmuls run underneath them.

Per-core device kernel (bf16 compute, f32 accumulation):
  - HOST-SIDE MASK COMPACTION: only the unmasked keys (max 1058 of 2048
    for these inputs) enter the K/V path, padded to NKV=1152; softmax over
    the compacted keys equals the masked softmax exactly. Pad rows have
    zero X (so zero scores) and exp bias -50, contributing ~2e-22.
  - Score/exp tiles cover ONE key chunk x TWO query blocks [128, 1024]:
    within a tile the mask depends only on the partition (the key), so
    the pad mask folds into the exp's per-partition bias AP and E comes
    out of ScalarE already masked, bf16. 18 activations total; no
    max-subtraction (|scores| <= ~1.5 by construction).
  - Softmax denominator: running TENSOR_TENSOR adds over the 9 pre-masked
    E tiles per block pair, one [K=128,M=1,N=512] matmul with a ones
    column, reciprocal_approx_fast (DVE), gpsimd partition_broadcast, and
    a final DVE multiply (f32 PSUM x f32 -> bf16).
  - Output written as OT [128, 2048] bf16; host transposes to [2048, 128]
    and upcasts to f32.
"""

import sys

for _p in ("/opt/trn_rl_repo", "/root/.axon_site/_ro/trn_rl_repo"):
    if _p not in sys.path:
        sys.path.append(_p)

import numpy as np
import ml_dtypes

B, S, DIN, DOUT = 8, 2048, 1024, 128
NCHUNK = DIN // 128          # 8 contraction chunks
NKV = 1152                   # padded compacted-key count (max kept + slack)
NKEY = NKV // 128            # 9 key chunks after host-side mask compaction
QBLK = 512                   # query block (free dim of S^T / OT matmuls)
NQB = S // QBLK              # 4 query blocks

BF16 = ml_dtypes.bfloat16
FP8E3 = ml_dtypes.float8_e3m4
WSCALE = 16.0                # fp8 weight scale (wk/wq shipped at 16x)
EXP_SCALE = float(1.0 / (WSCALE * np.sqrt(np.float32(S))))


def _build():
    import concourse.tile as tile
    from concourse import bacc, mybir
    from concourse.masks import make_identity

    f32 = mybir.dt.float32
    bf16 = mybir.dt.bfloat16
    fp8e3 = mybir.dt.float8e3
    Exp = mybir.ActivationFunctionType.Exp

    nc = bacc.Bacc("TRN2", target_bir_lowering=False, debug=False, num_devices=B)

    xkv_d = nc.dram_tensor("xkv", [DIN, NKV], bf16, kind="ExternalInput")
    # X^T for the Q path, fp8-e3m4, query-block-major [128, block, chunk,
    # 512] so QT block t unblocks as its transfer lands.
    xtq_d = nc.dram_tensor("xtq", [128, NQB * NCHUNK * QBLK], fp8e3,
                           kind="ExternalInput")
    # wk|wq packed fp8-e3m4 at 16x scale (one DMA); the 1/16 unfolds for
    # free: kt's Identity cast gets scale=1/16, Q's lands in the exp scale.
    w8_d = nc.dram_tensor("w8", [128, 2 * DIN], fp8e3, kind="ExternalInput")
    wv_d = nc.dram_tensor("wv", [128, DIN], bf16, kind="ExternalInput")
    # bcol (cols 0-3) and mb (cols 4-12) packed into one f32 blob.
    cst_d = nc.dram_tensor("cst", [128, 4 + NKEY], f32, kind="ExternalInput")
    out_d = nc.dram_tensor("out", [DOUT, S], bf16, kind="ExternalOutput")

    with tile.TileContext(nc) as tc:
        with (
            tc.tile_pool(name="persist", bufs=1) as pp,
            tc.tile_pool(name="epool", bufs=20) as ep,
            tc.tile_pool(name="tree", bufs=6) as tp,
            tc.tile_pool(name="normp", bufs=2) as rp,
            tc.tile_pool(name="outp", bufs=2) as op,
        ):
            # xkv DIN-chunks: c0 ships alone (small first transfer -> KV
            # matmuls start earliest), then pairs (1,2) (3,4) (5,6), then c7.
            # Five transfers stay under the ~8-semaphore rotation.
            xkv0 = pp.tile([128, NKV], bf16, tag="xkv0")
            xkvp = [pp.tile([128, 2, NKV], bf16, tag=f"xkvp{p}",
                            name=f"xkvp{p}") for p in range(3)]
            xkv7 = pp.tile([128, NKV], bf16, tag="xkv7")

            def xkv_sl(c, lo, hi):
                if c == 0:
                    return xkv0[:, lo:hi]
                if c == NCHUNK - 1:
                    return xkv7[:, lo:hi]
                return xkvp[(c - 1) // 2][:, (c - 1) % 2, lo:hi]
            xtq = pp.tile([128, NQB, NCHUNK, QBLK], fp8e3, tag="xtq")
            w8 = pp.tile([128, 2 * DIN], fp8e3, tag="w8")
            wv = pp.tile([128, DIN], bf16, tag="wv")
            cst = pp.tile([128, 4 + NKEY], f32, tag="cst")
            ocol = pp.tile([128, 1], bf16, tag="ocol")
            ident = pp.tile([128, 128], bf16, tag="ident")
            qt = pp.tile([128, S], bf16, tag="qt")
            kt = pp.tile([128, NKV], bf16, tag="kt")
            vt = pp.tile([128, NKV], bf16, tag="vt")
            vn = pp.tile([128, NKV], bf16, tag="vn")
            wrm_i = pp.tile([1, 32], f32, tag="wrm_i")
            wrm_o = pp.tile([1, 32], f32, tag="wrm_o")

            # exp table preload (overlaps the input DMA)
            nc.vector.memset(wrm_i[:], 0.0)
            nc.scalar.activation(wrm_o[:], wrm_i[:], Exp)

            nc.vector.memset(ocol[:], 1.0)
            make_identity(nc, ident[:])

            # PE warm-up scratch: dummy matmuls while the first xkv chunks
            # stream in flip the HAM clock gate to 8/8 early.
            wrm_l = pp.tile([128, 128], bf16, tag="wrm_l")
            wrm_r = pp.tile([128, QBLK], bf16, tag="wrm_r")
            nc.vector.memset(wrm_l[:], 0.0)
            nc.vector.memset(wrm_r[:], 0.0)

            # DMAs. Sync queue carries ONLY the xkv pairs (the critical KT/VT
            # gates) then the two late XT blocks; weights + the first two XT
            # blocks + consts ride the scalar queue in parallel. 11 transfers
            # total; the first wave of 8 fits the DMA semaphore rotation.
            xkv3 = xkv_d.ap().rearrange("(c p) m -> p c m", p=128)
            xtq4 = xtq_d.ap().rearrange("p (t c m) -> p t c m",
                                        t=NQB, c=NCHUNK)
            nc.sync.dma_start(xkv0[:], xkv3[:, 0, :])
            for p in range(3):
                nc.sync.dma_start(xkvp[p][:],
                                  xkv3[:, 2 * p + 1:2 * p + 3, :])
            nc.sync.dma_start(xkv7[:], xkv3[:, NCHUNK - 1, :])
            for t in (2, 3):
                nc.sync.dma_start(xtq[:, t, :, :], xtq4[:, t, :, :])
            nc.scalar.dma_start(w8[:], w8_d.ap())
            nc.scalar.dma_start(wv[:], wv_d.ap())
            for t in (0, 1):
                nc.scalar.dma_start(xtq[:, t, :, :], xtq4[:, t, :, :])
            nc.scalar.dma_start(cst[:], cst_d.ap())

            def cast_bias(dst_sl, src, col):
                nc.vector.tensor_scalar_add(dst_sl, src, cst[:, col:col + 1])

            egs = {}      # (pair, j) -> E tile [128, 1024] bf16, pre-masked
            rlast = {}    # pair -> latest running-sum tile [128, 1024] bf16
            rdbs = {}     # t -> broadcast reciprocal [128, 512] f32
            pots = {}     # t -> AV accumulator PSUM tile

            # ---------- pass 1 (under the XKV DMA shadow): KT + VT over the
            # compacted keys (3 sub-blocks 512/512/128), chunk by chunk as
            # the DMA lands, then QT blocks 0,1 from the fp8 XT blocks.
            KVS = ((0, 512), (512, 1024), (1024, 1152))
            Ident = mybir.ActivationFunctionType.Identity
            with tc.tile_pool(name="psKV", bufs=1, space="PSUM") as psKV, \
                 tc.tile_pool(name="psQ", bufs=2, space="PSUM") as psQ:
                psK = [psKV.tile([128, hi - lo], f32, tag=f"pk{i}",
                                 name=f"pk{i}") for i, (lo, hi) in enumerate(KVS)]
                psV = [psKV.tile([128, hi - lo], f32, tag=f"pv{i}",
                                 name=f"pv{i}") for i, (lo, hi) in enumerate(KVS)]
                for _ in range(11):
                    nc.tensor.matmul(psK[0][:], wrm_l[:], wrm_r[:],
                                     start=True, stop=True)
                for c in range(NCHUNK):
                    wks = w8[:, c * 128:(c + 1) * 128]
                    wvs = wv[:, c * 128:(c + 1) * 128]
                    st, sp = (c == 0), (c == NCHUNK - 1)
                    for i, (lo, hi) in enumerate(KVS):
                        nc.tensor.matmul(psK[i][:], wks, xkv_sl(c, lo, hi),
                                         start=st, stop=sp)
                        nc.tensor.matmul(psV[i][:], wvs, xkv_sl(c, lo, hi),
                                         start=st, stop=sp)
                    if c == 0:
                        # filler matmuls bridge the PE bubble while the
                        # first xkv pair is still in flight (keeps HAM at
                        # 8/8 through the gap; gated work queues behind).
                        wrm_q = psQ.tile([128, QBLK], f32, tag="pq",
                                         name="wrm_q")
                        for _ in range(6):
                            nc.tensor.matmul(wrm_q[:], wrm_l[:], wrm_r[:],
                                             start=True, stop=True)
                for i, (lo, hi) in enumerate(KVS):
                    nc.scalar.activation(kt[:, lo:hi], psK[i][:], Ident,
                                         bias=cst[:, 1:2], scale=1.0 / 16.0)
                for i, (lo, hi) in enumerate(KVS):
                    cast_bias(vt[:, lo:hi], psV[i][:], 2)

                # QT blocks 0,1: fp8-e3m4 Wq (stationary, 16x) x fp8 XT.
                # Block 1 first with its DVE cast, block 0 second with a
                # ScalarE cast: the two casts run on PARALLEL engines, so
                # the last score matmul is gated by max(cast) instead of
                # their 1.3us serial sum.
                for t in (1, 0):
                    pq = psQ.tile([128, QBLK], f32, tag="pq", name=f"pq{t}")
                    for c in range(NCHUNK):
                        nc.tensor.matmul(
                            pq[:],
                            w8[:, DIN + c * 128:DIN + (c + 1) * 128],
                            xtq[:, t, c, :],
                            start=(c == 0), stop=(c == NCHUNK - 1))
                    if t == 1:
                        cast_bias(qt[:, t * QBLK:(t + 1) * QBLK], pq[:], 0)
                    else:
                        nc.scalar.activation(
                            qt[:, t * QBLK:(t + 1) * QBLK], pq[:], Ident,
                            bias=cst[:, 0:1])

            # ---------- phase C pools (ps_st lives through the R region) ----
            with tc.tile_pool(name="ps_st", bufs=2, space="PSUM") as ps_st:

                def emit_score_pair(pair, j):
                    """pair 0 -> q blocks 0,1; pair 1 -> q blocks 2,3."""
                    pst = ps_st.tile([128, 2 * QBLK], f32, tag="st",
                                     name=f"pst{pair}_{j}")
                    ktj = kt[:, j * 128:(j + 1) * 128]
                    for half in (1, 0):
                        t = 2 * pair + half
                        nc.tensor.matmul(
                            pst[:, half * QBLK:(half + 1) * QBLK],
                            ktj, qt[:, t * QBLK:(t + 1) * QBLK],
                            start=True, stop=True,
                        )
                    eg = ep.tile([128, 2 * QBLK], bf16, tag="e",
                                 name=f"eg{pair}_{j}")
                    nc.scalar.activation(eg[:], pst[:], Exp,
                                         bias=cst[:, 4 + j:5 + j],
                                         scale=EXP_SCALE)
                    egs[(pair, j)] = eg

                def emit_tree(pair, j):
                    """running masked-E sum for a block pair (plain adds).
                    Pair 1 stops at chunk 8: the last chunk's contribution
                    goes straight into the pd matmul accumulation so only
                    one small matmul trails the final exp."""
                    if pair == 1 and j == NKEY - 1:
                        return
                    eg = egs[(pair, j)]
                    if j == 0:
                        rlast[pair] = eg
                        return
                    r = tp.tile([128, 2 * QBLK], bf16, tag="r",
                                name=f"r{pair}_{j}")
                    nc.vector.tensor_add(r[:], rlast[pair][:], eg[:])
                    rlast[pair] = r

                def emit_av(t, j):
                    if j == 0:
                        pots[t] = ps_o.tile([128, QBLK], f32, tag="o",
                                            name=f"pot{t}")
                    eg = egs[(t // 2, j)]
                    half = t % 2
                    nc.tensor.matmul(
                        pots[t][:],
                        vn[:, j * 128:(j + 1) * 128],
                        eg[:, half * QBLK:(half + 1) * QBLK],
                        start=(j == 0), stop=(j == NKEY - 1),
                    )

                def emit_pd(t, pd=None):
                    """denominator matmul + reciprocal + broadcast (early).
                    For the tail blocks a pre-built partial (pd=) lets the
                    rlast sum run BEFORE the last exp; only the eg(1,8)
                    stop-matmul trails it."""
                    half = t % 2
                    sl = slice(half * QBLK, (half + 1) * QBLK)
                    if pd is None:
                        pd = ps_m.tile([1, QBLK], f32, tag="d",
                                       name=f"pd{t}")
                        nc.tensor.matmul(pd[:], ocol[:],
                                         rlast[t // 2][:, sl],
                                         start=True, stop=(t < 2))
                    if t >= 2:
                        nc.tensor.matmul(pd[:], ocol[:],
                                         egs[(1, NKEY - 1)][:, sl],
                                         start=False, stop=True)
                    rdc = rp.tile([1, QBLK], f32, tag="rdc", name=f"rdc{t}")
                    nc.vector.reciprocal_approx_fast(rdc[:], pd[:])
                    rdb = rp.tile([128, QBLK], f32, tag="rdb", name=f"rdb{t}")
                    nc.gpsimd.partition_broadcast(rdb[:], rdc[:])
                    rdbs[t] = rdb

                def emit_finish(t):
                    osb = op.tile([128, QBLK], bf16, tag="osb", name=f"osb{t}")
                    nc.vector.tensor_mul(osb[:], pots[t][:], rdbs[t][:])
                    nc.sync.dma_start(
                        out_d.ap()[:, t * QBLK:(t + 1) * QBLK], osb[:])

                # ---- R region: the 9 V transposes + QT blocks 2,3,
                # interleaved 3 PE-ops per score pair (matches exp pace).
                # Transposes first (vt is ready early; xtq blocks 2,3 land
                # mid-stream). ----
                with (
                    tc.tile_pool(name="psA2", bufs=2, space="PSUM") as psA2,
                    tc.tile_pool(name="ps_tr", bufs=2, space="PSUM") as ps_tr,
                ):
                    r_ops = []

                    def proj8(t):
                        pr = psA2.tile([128, QBLK], f32, tag="pr",
                                       name=f"pr_q{t}")
                        for c in range(NCHUNK):
                            r_ops.append(lambda c=c, pr=pr, t=t:
                                nc.tensor.matmul(
                                    pr[:],
                                    w8[:, DIN + c * 128:DIN + (c + 1) * 128],
                                    xtq[:, t, c, :],
                                    start=(c == 0), stop=(c == NCHUNK - 1)))
                        r_ops.append(lambda pr=pr, t=t: cast_bias(
                            qt[:, t * QBLK:(t + 1) * QBLK], pr[:], 0))

                    def tr1(k):
                        ptr = ps_tr.tile([128, 128], bf16, tag="tr",
                                         name=f"tr{k}")
                        nc.tensor.transpose(
                            ptr[:], vt[:, k * 128:(k + 1) * 128], ident[:])
                        nc.vector.tensor_copy(
                            vn[:, k * 128:(k + 1) * 128], ptr[:])

                    for k in range(NKEY):
                        r_ops.append(lambda k=k: tr1(k))
                    proj8(2)
                    proj8(3)

                    ri = 0
                    for j in range(NKEY):
                        emit_score_pair(0, j)
                        emit_tree(0, j)
                        for _ in range(3):
                            if ri < len(r_ops):
                                r_ops[ri]()
                                ri += 1
                    while ri < len(r_ops):
                        r_ops[ri]()
                        ri += 1

                # ---- steady phase C ----
                # PSUM budget: ps_st 8KB + 3 concurrent pots 6KB + pd 2KB
                # = 16KB exactly, so AV for block 3 trails the loop.
                with (
                    tc.tile_pool(name="ps_o", bufs=3, space="PSUM") as ps_o,
                    tc.tile_pool(name="ps_m", bufs=1, space="PSUM") as ps_m,
                ):
                    # exp-paced: 5 matmuls per iteration (~1.08us) just under
                    # the 1.11us exp pace; block-3 AV trails (only 3 AV
                    # accumulators fit in PSUM next to the score tiles).
                    for j in range(NKEY):
                        emit_score_pair(1, j)
                        emit_av(0, j)
                        emit_av(1, j)
                        if j >= 2:
                            emit_av(2, j - 2)
                        if j == 1:
                            emit_pd(0)
                        if j == 2:
                            emit_pd(1)
                        if j == NKEY - 1:
                            # finish 0,1 BEFORE tree(1,8) hits the DVE
                            # queue, so mul(0) frees pot0's bank for pot3
                            # without waiting on the last exp.
                            emit_finish(0)
                            emit_finish(1)
                            # pd2's rlast partial runs pre-last-exp
                            pd2_pre = ps_m.tile([1, QBLK], f32, tag="d",
                                                name="pd2")
                            nc.tensor.matmul(pd2_pre[:], ocol[:],
                                             rlast[1][:, 0:QBLK],
                                             start=True, stop=False)
                        emit_tree(1, j)
                    # trailing: pd(2)/pd(3) chains (recip -> gpsimd bcast ->
                    # mul -> out DMA) are the critical path after the last
                    # exp, so their matmuls go FIRST; the block-3 AV matmuls
                    # (data ready, pot3 bank frees once finish(0)'s mul
                    # drains pot0) run underneath the chains.
                    emit_pd(2, pd=pd2_pre)
                    emit_av(2, NKEY - 2)
                    emit_av(2, NKEY - 1)
                    emit_pd(3)
                    for j in range(NKEY):
                        emit_av(3, j)
                    emit_finish(2)
                    emit_finish(3)

    nc.compile()
    return nc


_NC = None


def _get_nc():
    global _NC
    if _NC is None:
        _NC = _build()
    return _NC


def _prep_in_maps(input_tensor, attention_mask, Wq, bq, Wk, bk, Wv, bv):
    def pack_w(w, sc=None):
        w = np.asarray(w, np.float32)
        if sc is not None:
            w = w * sc
        # [1024, 128] -> [128, 8*128]: row c*128+p, col e -> [p, c*128+e]
        return np.ascontiguousarray(
            w.reshape(NCHUNK, 128, DOUT).transpose(1, 0, 2).reshape(128, DIN)
        )

    # wk | wq packed fp8-e3m4 at 16x (the 1/16 unfolds in kt's cast scale
    # and in the exp scale respectively; 1/sqrt(S) also rides the exp scale)
    w8_h = np.ascontiguousarray(np.concatenate(
        [pack_w(Wk, WSCALE), pack_w(Wq, WSCALE)], axis=1)).astype(FP8E3)
    wv_h = pack_w(Wv).astype(BF16)
    bcol_h = np.zeros((128, 4), np.float32)
    bcol_h[:, 0] = np.asarray(bq, np.float32) * WSCALE
    bcol_h[:, 1] = np.asarray(bk, np.float32)
    bcol_h[:, 2] = np.asarray(bv, np.float32)

    x = np.asarray(input_tensor, np.float32)
    m = np.asarray(attention_mask)
    in_maps = []
    for b in range(B):
        # X^T fp8-e3m4 for the Q path, laid out [128, t, c, 512]:
        # partition p, block t, chunk c, col j <- X^T[c*128+p, t*512+j].
        xt = np.ascontiguousarray(x[b].T)                           # [DIN, S]
        xtq_h = np.ascontiguousarray(
            xt.reshape(NCHUNK, 128, NQB, QBLK).transpose(1, 2, 0, 3)
            .reshape(128, NQB * NCHUNK * QBLK)).astype(FP8E3)
        # host-side mask compaction: keep only unmasked keys for the K/V
        # path, padded to NKV with zero rows; softmax over the compacted
        # keys equals the masked softmax. Pad rows get exp bias -50 (their
        # K is 0 so scores are 0 -> exp ~ 2e-22).
        keep = np.nonzero(m[b])[0]
        nk = len(keep)
        assert nk <= NKV, f"mask keeps {nk} keys > padded capacity {NKV}"
        xkv = np.zeros((NKV, DIN), np.float32)
        xkv[:nk] = x[b][keep]
        xkv_h = np.ascontiguousarray(xkv.T).astype(BF16)            # [DIN, NKV]
        mask_kv = np.zeros(NKV, np.float32)
        mask_kv[:nk] = 1.0
        mb_h = np.ascontiguousarray(
            (mask_kv.reshape(NKEY, 128).T - 1.0) * 50.0)            # [128, 9]
        cst_h = np.ascontiguousarray(
            np.concatenate([bcol_h, mb_h], axis=1))                 # [128, 13]
        in_maps.append({
            "xtq": xtq_h, "xkv": xkv_h, "w8": w8_h, "wv": wv_h, "cst": cst_h,
        })
    return in_maps


def run(in_maps, trace=False, **kwargs):
    from concourse.bass_utils import run_bass_kernel_spmd

    nc = _get_nc()
    return run_bass_kernel_spmd(
        nc, in_maps, core_ids=list(range(B)), trace=trace, **kwargs
    )


def kernel(input_tensor, attention_mask, Wq, bq, Wk, bk, Wv, bv):
    in_maps = _prep_in_maps(
        input_tensor, attention_mask, Wq, bq, Wk, bk, Wv, bv)
    res = run(in_maps, trace=False)
    out = np.stack([res.results[b]["out"].T for b in range(B)])
    return np.ascontiguousarray(out.astype(np.float32))
